# revision 45
# baseline (speedup 1.0000x reference)
"""Trainium2 Bass kernel for the Guided-Conv problem (v3, bf16 datapath).

Math (per independent sample n, of NB = 4096):
  g_n, d_n : 24x24x9 patches of guidance / depth.
  c_n      = conv2d(g_n, conv_w, stride 8, SAME) + conv_b        -> 3x3x9
  k_n[i]   = c_n[:, :, i] / max(||c_n[:, :, i]||_2, 1)           (per-channel 3x3 filter)
  gap_n    = mean(g_n, (y, x))                                   -> 9
  W2_n     = (gap_n @ dense_w + dense_b).reshape(9, 9)           (i2 -> o2)
  r2_n[o]  = 1 / max(||W2_n[:, o]||_2, 1)
  out_n    = (depthwise(d_n, k_n) @ W2_n) * r2_n                 -> 24x24x9

Device strategy (per core: 512 samples + 6 pad = 37 groups of 14):
  Partition layout q = n_local*9 + ch on 126 partitions; free = pixels.
  All bulk data is bf16 (halves HBM traffic, enables PE fast-weight-load
  and DVE 2x modes); norm math stays fp32.
  - The block-diagonal lhsT weights (kron(eye(14), w), 93% zeros) are NOT
    DMA'd: tiny compact tensors come over and are expanded on-chip with one
    masked-broadcast multiply each (mask row 126 is all-ones = bias row).
  - Kernel generation (c_n, W2_n) via block-diagonal matmuls, K=(n,ch),
    M padded to 128 cols so the PE fast-weight-load path engages.
  - Depthwise(3x3) + 1x1 fused: out[(n,o), pix] = sum_{t,i} BD_t[(n,i),(n,o)]
    * d_pad[(n,i), pix+t], 9 tap-matmuls accumulating in PSUM.
    BD_t = mask (.) (W2row-bcast) (.) k[:, t], built per-group on DVE from
    kw2[q,g,t,o] = k*W2 (2-byte 2x mode).
  - r2 applied as the per-partition ACT scale on the PSUM->SBUF copy.
  - All gin supergroups are DMA'd upfront (SBUF is big enough), split
    across the scalar and gpsimd rings; din streams on the sync ring.
  - tile_wait_until hints keep supergroup i's weight-gen placed inside
    supergroup i-1's main conv on the static engine queues.
Host does layout (patch extraction, de-interleave, zero-pad, fp32->bf16);
output returns bf16 and is upcast on host.
"""

import numpy as np
import ml_dtypes

import concourse.bass as bass
from concourse import bacc
import concourse.mybir as mybir
from concourse.tile import TileContext
from concourse.bass_utils import run_bass_kernel_spmd

F = 9          # channels
P = 24         # patch size
PADW = 26      # padded patch width (SAME conv, pad 1)
KS = 3         # generated kernel size
NCORES = 8
NL = 14        # samples per group
Q = NL * F     # 126 used partitions
QP = 128       # M-padded weight column count (FWL wants 128)
NGROUP = 37    # groups per core (36 full + 1 padded)
SPC = NGROUP * NL  # 518 sample slots per core (512 real)
PIX = P * P        # 576
PPIX = PADW * PADW  # 676
HALF = PIX // 2    # 288, pixels per PSUM chunk (<=512 fp32/bank)
SUPER = [2, 8, 9, 9, 9]  # weight-gen supertile sizes (sum = 37); ramp up
NGMAX = max(SUPER)
DQUAD = 4      # groups per din/out DMA

F32 = mybir.dt.float32
BF16 = mybir.dt.bfloat16
BF = ml_dtypes.bfloat16


def build_program():
    nc = bacc.Bacc("TRN2", target_bir_lowering=False, debug=False,
                   num_devices=NCORES)

    gin = nc.dram_tensor("gin", [Q, NGROUP, PIX], BF16, kind="ExternalInput").ap()
    din = nc.dram_tensor("din", [Q, NGROUP, PPIX], BF16, kind="ExternalInput").ap()
    # all small consts packed into one tensor = one DMA (tiny transfers pay
    # a large per-dma_start serialization cost on the rings)
    cpackd = nc.dram_tensor("cpack", [Q + 1, 3 * F * F + Q + 1], BF16,
                            kind="ExternalInput").ap()
    outd = nc.dram_tensor("out", [Q, NGROUP, PIX], BF16, kind="ExternalOutput").ap()

    with TileContext(nc) as tc:
        with (
            tc.tile_pool(name="consts", bufs=1) as cpool,
            tc.tile_pool(name="gpool", bufs=1) as gpool,
            tc.tile_pool(name="dpool", bufs=6) as dpool,
            tc.tile_pool(name="opool", bufs=4) as opool,
            tc.tile_pool(name="small", bufs=1) as spool,
            tc.tile_pool(name="gapp", bufs=2) as gappool,
            tc.tile_pool(name="bd", bufs=3) as bdpool,
            tc.tile_pool(name="ps_c", bufs=2, space="PSUM") as pcpool,
            tc.tile_pool(name="ps_d", bufs=2, space="PSUM") as pdpool,
            tc.tile_pool(name="ps_main", bufs=4, space="PSUM") as pmpool,
        ):
            # ---- persistent per-core small tensors ----
            craw = spool.tile([Q, NGROUP, F], F32, tag="craw")     # c + conv_b
            knorm = spool.tile([Q, NGROUP, F], F32, tag="knorm")   # normalized taps
            w2 = spool.tile([Q, NGROUP, F], BF16, tag="w2")        # raw W2 (D2 layout)
            kw2 = spool.tile([Q, NGROUP, F, F], BF16, tag="kw2")   # k (x) W2
            r2 = spool.tile([Q, NGROUP], F32, tag="r2")            # 1/max(n2,1)
            r1 = spool.tile([Q, NGROUP], F32, tag="r1")            # 1/max(n1,1)
            sq = spool.tile([Q, NGROUP, F], F32, tag="sq")         # scratch squares
            s1 = spool.tile([Q, NGROUP], F32, tag="s1")            # scratch sums

            # ---- consts: one packed DMA + convb, first on the scalar ring ----
            cpack_sb = cpool.tile([Q + 1, 3 * F * F + Q + 1], BF16, tag="cpack")
            nc.scalar.dma_start(out=cpack_sb[0:64], in_=cpackd[0:64])
            nc.sync.dma_start(out=cpack_sb[64:Q + 1], in_=cpackd[64:Q + 1])
            convb_sb = cpack_sb[0:Q, 3 * F * F + Q:]
            lhsAc_sb = cpack_sb[0:Q, 0:F * F].rearrange(
                "p (t c) -> p t c", c=F)
            lhsDc_sb = cpack_sb[:, F * F:2 * F * F].rearrange(
                "p (t c) -> p t c", c=F)
            lhsD2c_sb = cpack_sb[:, 2 * F * F:3 * F * F].rearrange(
                "p (t c) -> p t c", c=F)
            maskE_sb = cpack_sb[:, 3 * F * F:3 * F * F + Q].rearrange(
                "p (a b) -> p a b", b=F)
            mask_sb = maskE_sb[0:Q]

            # ---- all gin supergroups upfront, split over two rings ----
            gsbs = []
            g0 = 0
            for si, ng in enumerate(SUPER):
                gsb = gpool.tile([Q, ng * PIX], BF16, tag=f"gsb{si}")
                nc.gpsimd.dma_start(out=gsb,
                              in_=gin[:, g0:g0 + ng].rearrange("p g f -> p (g f)"))
                gsbs.append(gsb)
                g0 += ng

            # warm the ACT table (sqrt set also holds copy/identity/square)
            # while the first DMAs are in flight (after the descriptor-gen
            # instructions above, so it doesn't delay them).
            warm = spool.tile([Q, 1], F32, tag="warm")
            nc.vector.memset(warm, 1.0)
            nc.scalar.sqrt(out=warm, in_=warm)

            # ---- expand block-diagonal lhsT weights on-chip ----
            # lhs[q, t, n'*9+c] = compact[q, t, c] * maskE[q, n', c]
            # (maskE row 126 = all-ones: the dense bias row reaches every
            # sample block). Pad cols 126..127 with zeros for 128-col FWL.
            lhsA_sb = cpool.tile([Q, KS * KS, QP], BF16, tag="lhsA")
            lhsD_sb = cpool.tile([Q + 1, F, QP], BF16, tag="lhsD")
            lhsD2_sb = cpool.tile([Q + 1, F, QP], BF16, tag="lhsD2")
            nc.vector.memset(lhsA_sb, 0.0)
            nc.vector.memset(lhsD_sb, 0.0)
            nc.vector.memset(lhsD2_sb, 0.0)
            nc.vector.tensor_mul(
                out=lhsA_sb[:, :, 0:Q].rearrange("p t (a b) -> p t a b", b=F),
                in0=lhsAc_sb.unsqueeze(2).broadcast_to([Q, KS * KS, NL, F]),
                in1=mask_sb.unsqueeze(1).broadcast_to([Q, KS * KS, NL, F]))
            nc.vector.tensor_mul(
                out=lhsD_sb[:, :, 0:Q].rearrange("p t (a b) -> p t a b", b=F),
                in0=lhsDc_sb.unsqueeze(2).broadcast_to([Q + 1, F, NL, F]),
                in1=maskE_sb.unsqueeze(1).broadcast_to([Q + 1, F, NL, F]))
            nc.vector.tensor_mul(
                out=lhsD2_sb[:, :, 0:Q].rearrange("p t (a b) -> p t a b", b=F),
                in0=lhsD2c_sb.unsqueeze(2).broadcast_to([Q + 1, F, NL, F]),
                in1=maskE_sb.unsqueeze(1).broadcast_to([Q + 1, F, NL, F]))

            # Scheduler placement hints: weight-gen for supergroup i belongs
            # inside supergroup i-1's main conv on the static engine queues.
            est_main_start = []
            t = 0.023  # ms, estimated first main-conv MM
            for ng in SUPER:
                est_main_start.append(t)
                t += ng * 0.0028  # ~2.8us per group

            g0 = 0
            for si, ng in enumerate(SUPER):
                gsl = slice(g0, g0 + ng)
                gsb = gsbs[si]
                wg_wait = est_main_start[si - 2] + 0.001 if si >= 2 else 0.0
                ctx_wg = tc.tile_wait_until(wg_wait, enable=si >= 2)
                ctx_wg.__enter__()

                # ---------- weight generation for ng groups ----------
                # step A: strided conv -> c, 9 accumulated BD matmuls
                psc = pcpool.tile([QP, ng, F], F32, tag="psc")
                gwin = gsb.rearrange(
                    "p (g oy yr ox xr) -> p g oy ox yr xr",
                    g=ng, oy=KS, yr=8, ox=KS, xr=8)
                for t_ in range(KS * KS):
                    ky, kx = divmod(t_, KS)
                    nc.tensor.matmul(
                        psc,
                        lhsT=lhsA_sb[:, t_, :],
                        rhs=gwin[:, :, :, :, ky, kx],
                        start=(t_ == 0), stop=(t_ == KS * KS - 1),
                        skip_group_check=True)

                # craw = psc + conv_b (per-partition bias)
                nc.scalar.activation(
                    out=craw[:, gsl, :], in_=psc[0:Q],
                    func=mybir.ActivationFunctionType.Identity,
                    bias=convb_sb, scale=1.0)

                # gap: per-group pixel SUM, bf16 out (one rounding; the 1/576
                # mean scale is folded into lhsD/lhsD2 on the host). Row 126
                # reads 1.0 for the bias row of the K=127 dense matmuls.
                gap = gappool.tile([QP, ng], BF16, tag="gap")
                nc.vector.memset(gap, 1.0)
                with nc.allow_low_precision("gap sum rounds once at bf16 "
                                            "write; tolerance is 2e-2"):
                    for gi in range(ng):
                        nc.vector.tensor_reduce(
                            out=gap[0:Q, gi:gi + 1],
                            in_=gsb[:, gi * PIX:(gi + 1) * PIX],
                            axis=mybir.AxisListType.X, op=mybir.AluOpType.add)

                # dense layer, both layouts (D for the norm, D2 for the values)
                psD12 = pdpool.tile([QP, 2, F, ng], F32, tag="psD12")
                psD = psD12[:, 0]
                psD2 = psD12[:, 1]
                for j in range(F):
                    nc.tensor.matmul(psD[:, j, :], lhsT=lhsD_sb[:, j, :],
                                     rhs=gap[0:Q + 1, :],
                                     start=True, stop=True,
                                     skip_group_check=True)
                for j in range(F):
                    nc.tensor.matmul(psD2[:, j, :], lhsT=lhsD2_sb[:, j, :],
                                     rhs=gap[0:Q + 1, :],
                                     start=True, stop=True,
                                     skip_group_check=True)

                # r2 = 1/max(||W2[:,o]||, 1):  sum_i2 D^2 per (n,o2)
                nc.scalar.square(out=sq[:, gsl, :],
                                 in_=psD[0:Q].rearrange("p i g -> p g i"))
                nc.vector.tensor_reduce(
                    out=s1[:, gsl], in_=sq[:, gsl, :],
                    axis=mybir.AxisListType.X, op=mybir.AluOpType.add)
                nc.scalar.sqrt(out=s1[:, gsl], in_=s1[:, gsl])
                nc.vector.tensor_scalar_max(r2[:, gsl], s1[:, gsl], 1.0)
                nc.vector.reciprocal(r2[:, gsl], r2[:, gsl])

                # W2 raw values, group-major, bf16
                nc.scalar.copy(out=w2[:, gsl, :],
                               in_=psD2[0:Q].rearrange("p o g -> p g o"))

                # r1 = 1/max(||c||, 1) per (n, ch); knorm = craw * r1
                nc.vector.tensor_mul(
                    out=sq[:, gsl, :], in0=craw[:, gsl, :], in1=craw[:, gsl, :])
                nc.vector.tensor_reduce(
                    out=r1[:, gsl], in_=sq[:, gsl, :],
                    axis=mybir.AxisListType.X, op=mybir.AluOpType.add)
                nc.scalar.sqrt(out=r1[:, gsl], in_=r1[:, gsl])
                nc.vector.tensor_scalar_max(r1[:, gsl], r1[:, gsl], 1.0)
                nc.vector.reciprocal(r1[:, gsl], r1[:, gsl])
                nc.vector.tensor_mul(
                    out=knorm[:, gsl, :], in0=craw[:, gsl, :],
                    in1=r1[:, gsl].unsqueeze(2).broadcast_to([Q, ng, F]))

                # kw2[q, g, t, o] = knorm[q, g, t] * w2[q, g, o]
                nc.vector.tensor_mul(
                    out=kw2[:, gsl, :, :],
                    in0=knorm[:, gsl, :].unsqueeze(3).broadcast_to([Q, ng, F, F]),
                    in1=w2[:, gsl, :].unsqueeze(2).broadcast_to([Q, ng, F, F]))

                # bd[q, g, t, (n', o)] = kw2[q, g, t, o] * mask[q, n', o]
                # per-group 2-byte 2x DVE ops; cols 126-127 stay garbage
                # (pad for the 128-col FWL weight loads, never read).
                bd = bdpool.tile([Q, NGMAX, KS * KS, QP], BF16, tag="bd")
                for gi in range(ng):
                    nc.vector.tensor_mul(
                        out=bd[:, gi, :, 0:Q].rearrange(
                            "p t (a b) -> p t a b", b=F),
                        in0=kw2[:, g0 + gi, :, :].unsqueeze(2).broadcast_to(
                            [Q, F, NL, F]),
                        in1=mask_sb.unsqueeze(1).broadcast_to([Q, F, NL, F]))
                ctx_wg.__exit__(None, None, None)

                # ---------- main conv, groups in quads (fewer DMA setups) ----
                for p0 in range(g0, g0 + ng, DQUAD):
                    nq = min(DQUAD, g0 + ng - p0)
                    dsb = dpool.tile([Q, DQUAD * PPIX], BF16, tag="dsb")
                    dv = din[:, p0:p0 + nq].rearrange("p g f -> p (g f)")
                    if p0 == 0:
                        # quad 0 gates the first main-conv matmul; split it
                        # across the sync and scalar rings so it lands ~7us
                        # earlier than queued whole behind cpack's half
                        nc.sync.dma_start(out=dsb[0:64, :nq * PPIX],
                                          in_=dv[0:64])
                        nc.scalar.dma_start(out=dsb[64:Q, :nq * PPIX],
                                            in_=dv[64:Q])
                    else:
                        nc.sync.dma_start(out=dsb[:, :nq * PPIX], in_=dv)
                    osb = opool.tile([Q, DQUAD * PIX], BF16, tag="osb")

                    for gl in range(nq):
                        g = p0 + gl
                        gi = g - g0
                        drows = dsb[:, gl * PPIX:(gl + 1) * PPIX].rearrange(
                            "p (r c) -> p r c", c=PADW)
                        pm0 = pmpool.tile([QP, HALF], F32, tag="pm")
                        pm1 = pmpool.tile([QP, HALF], F32, tag="pm")
                        pms = [pm0, pm1]
                        for t_ in range(KS * KS):
                            ky, kx = divmod(t_, KS)
                            lhsT = bd[:, gi, t_, :]
                            for h in range(2):
                                rhs = drows[:, h * 12 + ky:h * 12 + ky + 12,
                                            kx:kx + P]
                                nc.tensor.matmul(
                                    pms[h], lhsT=lhsT, rhs=rhs,
                                    start=(t_ == 0), stop=(t_ == KS * KS - 1),
                                    skip_group_check=True)

                        for h in range(2):
                            nc.scalar.activation(
                                out=osb[:, gl * PIX + h * HALF:
                                        gl * PIX + (h + 1) * HALF],
                                in_=pms[h][0:Q],
                                func=mybir.ActivationFunctionType.Copy,
                                bias=0.0, scale=r2[:, g:g + 1])
                    oeng = nc.gpsimd if (p0 // DQUAD) % 2 == 0 else nc.sync
                    oeng.dma_start(
                        out=outd[:, p0:p0 + nq].rearrange("p g f -> p (g f)"),
                        in_=osb[:, :nq * PIX])

                g0 += ng

    nc.compile()
    return nc


def _host_prep(guidance, depth, conv_w, conv_b, dense_w, dense_b):
    B, H, W, _ = guidance.shape
    nh, nw = H // P, W // P
    NB = B * nh * nw

    def to_samples(x):
        # (B,H,W,F) -> (NB, P, P, F), sample order = flat (b, i, j)
        return (x.reshape(B, nh, P, nw, P, F)
                 .transpose(0, 1, 3, 2, 4, 5)
                 .reshape(NB, P, P, F))

    gs = to_samples(np.ascontiguousarray(guidance))
    ds = to_samples(np.ascontiguousarray(depth))

    in_maps = []
    for c in range(NCORES):
        gsl = gs[c * 512:(c + 1) * 512]
        dsl = ds[c * 512:(c + 1) * 512]
        gpad = np.zeros((SPC, P, P, F), BF)
        gpad[:512] = gsl
        dpad = np.zeros((SPC, PADW, PADW, F), BF)
        dpad[:512, 1:P + 1, 1:P + 1] = dsl
        # (SPC, y, x, ch) -> [NGROUP, 126, pix]  with q = n_local*9 + ch
        gq = (gpad.reshape(NGROUP, NL, P, P, F)
                  .transpose(1, 4, 0, 2, 3)
                  .reshape(Q, NGROUP, PIX))
        dq = (dpad.reshape(NGROUP, NL, PADW, PADW, F)
                  .transpose(1, 4, 0, 2, 3)
                  .reshape(Q, NGROUP, PPIX))
        in_maps.append({"gin": np.ascontiguousarray(gq),
                        "din": np.ascontiguousarray(dq)})

    # compact per-partition weights; block-diag expansion happens on-chip
    iofq = np.arange(Q) % F                       # i(q)
    lhsAc = np.ascontiguousarray(
        conv_w.reshape(KS * KS, F, F)[:, iofq, :].transpose(1, 0, 2)).astype(BF)
    dws = dense_w.astype(np.float32) / PIX  # gap arrives as a SUM over pixels
    lhsDc = np.zeros((Q + 1, F, F), np.float32)
    lhsD2c = np.zeros((Q + 1, F, F), np.float32)
    for j in range(F):
        lhsDc[:Q, j, :] = dws[iofq][:, j * F:(j + 1) * F]
        lhsDc[Q, j, :] = dense_b[j * F:(j + 1) * F]
        lhsD2c[:Q, j, :] = dws[iofq][:, j::F]
        lhsD2c[Q, j, :] = dense_b[j::F]
    maskE = np.zeros((Q + 1, Q), np.float32)
    maskE[:Q] = np.kron(np.eye(NL, dtype=np.float32), np.ones((F, F), np.float32))
    maskE[Q] = 1.0
    convb = np.tile(conv_b.astype(np.float32), NL)[:, None]

    cpack = np.zeros((Q + 1, 3 * F * F + Q + 1), np.float32)
    cpack[:Q, 0:F * F] = np.asarray(lhsAc, np.float32).reshape(Q, F * F)
    cpack[:, F * F:2 * F * F] = lhsDc.reshape(Q + 1, F * F)
    cpack[:, 2 * F * F:3 * F * F] = lhsD2c.reshape(Q + 1, F * F)
    cpack[:, 3 * F * F:3 * F * F + Q] = maskE
    cpack[:Q, 3 * F * F + Q] = convb[:, 0]
    consts = {"cpack": np.ascontiguousarray(cpack.astype(BF))}
    for m in in_maps:
        m.update(consts)
    return in_maps


_CACHED_NC = None


def run(inputs, trace=False, **kw):
    """Build (cached), run on 8 cores, return (full_output, BassKernelResults)."""
    global _CACHED_NC
    inputs = {k: np.asarray(v, np.float32) for k, v in inputs.items()}
    in_maps = _host_prep(**inputs)
    if _CACHED_NC is None:
        _CACHED_NC = build_program()
    res = run_bass_kernel_spmd(_CACHED_NC, in_maps, list(range(NCORES)),
                               trace=trace, **kw)
    outs = []
    for c in range(NCORES):
        o = res.results[c]["out"].astype(np.float32)
        o = o.reshape(NL, F, NGROUP, P, P)
        o = o.transpose(2, 0, 3, 4, 1).reshape(SPC, P, P, F)[:512]
        outs.append(o)
    full = np.concatenate(outs, 0)  # (4096, 24, 24, 9) in (b, i, j) order
    B, H, W = 16, 384, 384
    return full.reshape(B, H, W, F), res


def kernel(**inputs):
    out, _ = run(inputs, trace=False)
    return out


# revision 46
# speedup vs baseline: 1.2519x; 1.2519x over previous
"""Trainium2 Bass kernel for the Guided-Conv problem (v3, bf16 datapath).

Math (per independent sample n, of NB = 4096):
  g_n, d_n : 24x24x9 patches of guidance / depth.
  c_n      = conv2d(g_n, conv_w, stride 8, SAME) + conv_b        -> 3x3x9
  k_n[i]   = c_n[:, :, i] / max(||c_n[:, :, i]||_2, 1)           (per-channel 3x3 filter)
  gap_n    = mean(g_n, (y, x))                                   -> 9
  W2_n     = (gap_n @ dense_w + dense_b).reshape(9, 9)           (i2 -> o2)
  r2_n[o]  = 1 / max(||W2_n[:, o]||_2, 1)
  out_n    = (depthwise(d_n, k_n) @ W2_n) * r2_n                 -> 24x24x9

Device strategy (per core: 512 samples + 6 pad = 37 groups of 14):
  Partition layout q = n_local*9 + ch on 126 partitions; free = pixels.
  All bulk data is bf16 (halves HBM traffic, enables PE fast-weight-load
  and DVE 2x modes); norm math stays fp32.
  - The block-diagonal lhsT weights (kron(eye(14), w), 93% zeros) are NOT
    DMA'd: tiny compact tensors come over and are expanded on-chip with one
    masked-broadcast multiply each (mask row 126 is all-ones = bias row).
  - Kernel generation (c_n, W2_n) via block-diagonal matmuls, K=(n,ch),
    M padded to 128 cols so the PE fast-weight-load path engages.
  - Depthwise(3x3) + 1x1 fused: out[(n,o), pix] = sum_{t,i} BD_t[(n,i),(n,o)]
    * d_pad[(n,i), pix+t], 9 tap-matmuls accumulating in PSUM.
    BD_t = mask (.) (W2row-bcast) (.) k[:, t], built per-group on DVE from
    kw2[q,g,t,o] = k*W2 (2-byte 2x mode).
  - r2 applied as the per-partition ACT scale on the PSUM->SBUF copy.
  - All gin supergroups are DMA'd upfront (SBUF is big enough), split
    across the scalar and gpsimd rings; din streams on the sync ring.
  - tile_wait_until hints keep supergroup i's weight-gen placed inside
    supergroup i-1's main conv on the static engine queues.
Host does layout (patch extraction, de-interleave, zero-pad, fp32->bf16);
output returns bf16 and is upcast on host.
"""

import numpy as np
import ml_dtypes

import concourse.bass as bass
from concourse import bacc
import concourse.mybir as mybir
from concourse.tile import TileContext
from concourse.bass_utils import run_bass_kernel_spmd

F = 9          # channels
P = 24         # patch size
PADW = 26      # padded patch width (SAME conv, pad 1)
KS = 3         # generated kernel size
NCORES = 8
NL = 14        # samples per group
Q = NL * F     # 126 used partitions
QP = 128       # M-padded weight column count (FWL wants 128)
NGROUP = 37    # groups per core (36 full + 1 padded)
SPC = NGROUP * NL  # 518 sample slots per core (512 real)
PIX = P * P        # 576
PPIX = PADW * PADW  # 676
HALF = PIX // 2    # 288, pixels per PSUM chunk (<=512 fp32/bank)
SUPER = [2, 8, 9, 9, 9]  # weight-gen supertile sizes (sum = 37); ramp up
NGMAX = max(SUPER)
DQUAD = 4      # groups per din/out DMA

F32 = mybir.dt.float32
BF16 = mybir.dt.bfloat16
BF = ml_dtypes.bfloat16


def build_program():
    nc = bacc.Bacc("TRN2", target_bir_lowering=False, debug=False,
                   num_devices=NCORES)

    gin = nc.dram_tensor("gin", [Q, NGROUP, PIX], BF16, kind="ExternalInput").ap()
    din = nc.dram_tensor("din", [Q, NGROUP, PPIX], BF16, kind="ExternalInput").ap()
    # all small consts packed into one tensor = one DMA (tiny transfers pay
    # a large per-dma_start serialization cost on the rings)
    cpackd = nc.dram_tensor("cpack", [Q + 1, 3 * F * F + Q + 1], BF16,
                            kind="ExternalInput").ap()
    outd = nc.dram_tensor("out", [Q, NGROUP, PIX], BF16, kind="ExternalOutput").ap()

    with TileContext(nc) as tc:
        with (
            tc.tile_pool(name="consts", bufs=1) as cpool,
            tc.tile_pool(name="gpool", bufs=1) as gpool,
            tc.tile_pool(name="dpool", bufs=6) as dpool,
            tc.tile_pool(name="opool", bufs=4) as opool,
            tc.tile_pool(name="small", bufs=1) as spool,
            tc.tile_pool(name="gapp", bufs=2) as gappool,
            tc.tile_pool(name="bd", bufs=3) as bdpool,
            tc.tile_pool(name="ps_c", bufs=2, space="PSUM") as pcpool,
            tc.tile_pool(name="ps_d", bufs=2, space="PSUM") as pdpool,
            tc.tile_pool(name="ps_main", bufs=4, space="PSUM") as pmpool,
        ):
            # ---- persistent per-core small tensors ----
            craw = spool.tile([Q, NGROUP, F], F32, tag="craw")     # c + conv_b
            knorm = spool.tile([Q, NGROUP, F], F32, tag="knorm")   # normalized taps
            w2 = spool.tile([Q, NGROUP, F], BF16, tag="w2")        # raw W2 (D2 layout)
            kw2 = spool.tile([Q, NGROUP, F, F], BF16, tag="kw2")   # k (x) W2
            r2 = spool.tile([Q, NGROUP], F32, tag="r2")            # 1/max(n2,1)
            r1 = spool.tile([Q, NGROUP], F32, tag="r1")            # 1/max(n1,1)
            sq = spool.tile([Q, NGROUP, F], F32, tag="sq")         # scratch squares
            s1 = spool.tile([Q, NGROUP], F32, tag="s1")            # scratch sums

            # ---- consts: one packed DMA + convb, first on the scalar ring ----
            cpack_sb = cpool.tile([Q + 1, 3 * F * F + Q + 1], BF16, tag="cpack")
            nc.scalar.dma_start(out=cpack_sb[0:64], in_=cpackd[0:64])
            nc.sync.dma_start(out=cpack_sb[64:Q + 1], in_=cpackd[64:Q + 1])
            convb_sb = cpack_sb[0:Q, 3 * F * F + Q:]
            lhsAc_sb = cpack_sb[0:Q, 0:F * F].rearrange(
                "p (t c) -> p t c", c=F)
            lhsDc_sb = cpack_sb[:, F * F:2 * F * F].rearrange(
                "p (t c) -> p t c", c=F)
            lhsD2c_sb = cpack_sb[:, 2 * F * F:3 * F * F].rearrange(
                "p (t c) -> p t c", c=F)
            maskE_sb = cpack_sb[:, 3 * F * F:3 * F * F + Q].rearrange(
                "p (a b) -> p a b", b=F)
            mask_sb = maskE_sb[0:Q]

            # ---- all gin supergroups upfront, split over two rings ----
            gsbs = []
            g0 = 0
            for si, ng in enumerate(SUPER):
                gsb = gpool.tile([Q, ng * PIX], BF16, tag=f"gsb{si}")
                nc.gpsimd.dma_start(out=gsb,
                              in_=gin[:, g0:g0 + ng].rearrange("p g f -> p (g f)"))
                gsbs.append(gsb)
                g0 += ng

            # warm the ACT table (sqrt set also holds copy/identity/square)
            # while the first DMAs are in flight (after the descriptor-gen
            # instructions above, so it doesn't delay them).
            warm = spool.tile([Q, 1], F32, tag="warm")
            nc.vector.memset(warm, 1.0)
            nc.scalar.sqrt(out=warm, in_=warm)

            # ---- expand block-diagonal lhsT weights on-chip ----
            # lhs[q, t, n'*9+c] = compact[q, t, c] * maskE[q, n', c]
            # (maskE row 126 = all-ones: the dense bias row reaches every
            # sample block). Pad cols 126..127 with zeros for 128-col FWL.
            lhsA_sb = cpool.tile([Q, KS * KS, QP], BF16, tag="lhsA")
            lhsD_sb = cpool.tile([Q + 1, F, QP], BF16, tag="lhsD")
            lhsD2_sb = cpool.tile([Q + 1, F, QP], BF16, tag="lhsD2")
            nc.vector.memset(lhsA_sb, 0.0)
            nc.vector.memset(lhsD_sb, 0.0)
            nc.vector.memset(lhsD2_sb, 0.0)
            nc.vector.tensor_mul(
                out=lhsA_sb[:, :, 0:Q].rearrange("p t (a b) -> p t a b", b=F),
                in0=lhsAc_sb.unsqueeze(2).broadcast_to([Q, KS * KS, NL, F]),
                in1=mask_sb.unsqueeze(1).broadcast_to([Q, KS * KS, NL, F]))
            nc.vector.tensor_mul(
                out=lhsD_sb[:, :, 0:Q].rearrange("p t (a b) -> p t a b", b=F),
                in0=lhsDc_sb.unsqueeze(2).broadcast_to([Q + 1, F, NL, F]),
                in1=maskE_sb.unsqueeze(1).broadcast_to([Q + 1, F, NL, F]))
            nc.vector.tensor_mul(
                out=lhsD2_sb[:, :, 0:Q].rearrange("p t (a b) -> p t a b", b=F),
                in0=lhsD2c_sb.unsqueeze(2).broadcast_to([Q + 1, F, NL, F]),
                in1=maskE_sb.unsqueeze(1).broadcast_to([Q + 1, F, NL, F]))

            # Scheduler placement hints: weight-gen for supergroup i belongs
            # inside supergroup i-1's main conv on the static engine queues.
            est_main_start = []
            t = 0.023  # ms, estimated first main-conv MM
            for ng in SUPER:
                est_main_start.append(t)
                t += ng * 0.0028  # ~2.8us per group

            g0 = 0
            for si, ng in enumerate(SUPER):
                gsl = slice(g0, g0 + ng)
                gsb = gsbs[si]
                wg_wait = est_main_start[si - 2] + 0.001 if si >= 2 else 0.0
                ctx_wg = tc.tile_wait_until(wg_wait, enable=si >= 2)
                ctx_wg.__enter__()

                # ---------- weight generation for ng groups ----------
                # step A: strided conv -> c, 9 accumulated BD matmuls
                psc = pcpool.tile([QP, ng, F], F32, tag="psc")
                gwin = gsb.rearrange(
                    "p (g oy yr ox xr) -> p g oy ox yr xr",
                    g=ng, oy=KS, yr=8, ox=KS, xr=8)
                for t_ in range(KS * KS):
                    ky, kx = divmod(t_, KS)
                    nc.tensor.matmul(
                        psc,
                        lhsT=lhsA_sb[:, t_, :],
                        rhs=gwin[:, :, :, :, ky, kx],
                        start=(t_ == 0), stop=(t_ == KS * KS - 1),
                        skip_group_check=True)

                # craw = psc + conv_b (per-partition bias)
                nc.scalar.activation(
                    out=craw[:, gsl, :], in_=psc[0:Q],
                    func=mybir.ActivationFunctionType.Identity,
                    bias=convb_sb, scale=1.0)

                # gap: per-group pixel SUM, bf16 out (one rounding; the 1/576
                # mean scale is folded into lhsD/lhsD2 on the host). Row 126
                # reads 1.0 for the bias row of the K=127 dense matmuls.
                gap = gappool.tile([QP, ng], BF16, tag="gap")
                nc.vector.memset(gap, 1.0)
                with nc.allow_low_precision("gap sum rounds once at bf16 "
                                            "write; tolerance is 2e-2"):
                    for gi in range(ng):
                        nc.vector.tensor_reduce(
                            out=gap[0:Q, gi:gi + 1],
                            in_=gsb[:, gi * PIX:(gi + 1) * PIX],
                            axis=mybir.AxisListType.X, op=mybir.AluOpType.add)

                # dense layer, both layouts (D for the norm, D2 for the values)
                psD12 = pdpool.tile([QP, 2, F, ng], F32, tag="psD12")
                psD = psD12[:, 0]
                psD2 = psD12[:, 1]
                for j in range(F):
                    nc.tensor.matmul(psD[:, j, :], lhsT=lhsD_sb[:, j, :],
                                     rhs=gap[0:Q + 1, :],
                                     start=True, stop=True,
                                     skip_group_check=True)
                for j in range(F):
                    nc.tensor.matmul(psD2[:, j, :], lhsT=lhsD2_sb[:, j, :],
                                     rhs=gap[0:Q + 1, :],
                                     start=True, stop=True,
                                     skip_group_check=True)

                # r2 = 1/max(||W2[:,o]||, 1):  sum_i2 D^2 per (n,o2)
                nc.scalar.square(out=sq[:, gsl, :],
                                 in_=psD[0:Q].rearrange("p i g -> p g i"))
                nc.vector.tensor_reduce(
                    out=s1[:, gsl], in_=sq[:, gsl, :],
                    axis=mybir.AxisListType.X, op=mybir.AluOpType.add)
                nc.scalar.sqrt(out=s1[:, gsl], in_=s1[:, gsl])
                nc.vector.tensor_scalar_max(r2[:, gsl], s1[:, gsl], 1.0)
                nc.vector.reciprocal(r2[:, gsl], r2[:, gsl])

                # W2 raw values, group-major, bf16
                nc.scalar.copy(out=w2[:, gsl, :],
                               in_=psD2[0:Q].rearrange("p o g -> p g o"))

                # r1 = 1/max(||c||, 1) per (n, ch); knorm = craw * r1
                nc.vector.tensor_mul(
                    out=sq[:, gsl, :], in0=craw[:, gsl, :], in1=craw[:, gsl, :])
                nc.vector.tensor_reduce(
                    out=r1[:, gsl], in_=sq[:, gsl, :],
                    axis=mybir.AxisListType.X, op=mybir.AluOpType.add)
                nc.scalar.sqrt(out=r1[:, gsl], in_=r1[:, gsl])
                nc.vector.tensor_scalar_max(r1[:, gsl], r1[:, gsl], 1.0)
                nc.vector.reciprocal(r1[:, gsl], r1[:, gsl])
                nc.vector.tensor_mul(
                    out=knorm[:, gsl, :], in0=craw[:, gsl, :],
                    in1=r1[:, gsl].unsqueeze(2).broadcast_to([Q, ng, F]))

                # kw2[q, g, t, o] = knorm[q, g, t] * w2[q, g, o]
                nc.vector.tensor_mul(
                    out=kw2[:, gsl, :, :],
                    in0=knorm[:, gsl, :].unsqueeze(3).broadcast_to([Q, ng, F, F]),
                    in1=w2[:, gsl, :].unsqueeze(2).broadcast_to([Q, ng, F, F]))

                # bd[q, g, t, (n', o)] = kw2[q, g, t, o] * mask[q, n', o]
                # per-group 2-byte 2x DVE ops; cols 126-127 stay garbage
                # (pad for the 128-col FWL weight loads, never read).
                bd = bdpool.tile([Q, NGMAX, KS * KS, QP], BF16, tag="bd")
                for gi in range(ng):
                    nc.vector.tensor_mul(
                        out=bd[:, gi, :, 0:Q].rearrange(
                            "p t (a b) -> p t a b", b=F),
                        in0=kw2[:, g0 + gi, :, :].unsqueeze(2).broadcast_to(
                            [Q, F, NL, F]),
                        in1=mask_sb.unsqueeze(1).broadcast_to([Q, F, NL, F]))
                ctx_wg.__exit__(None, None, None)

                # ---------- main conv, groups in quads (fewer DMA setups) ----
                for p0 in range(g0, g0 + ng, DQUAD):
                    nq = min(DQUAD, g0 + ng - p0)
                    dsb = dpool.tile([Q, DQUAD * PPIX], BF16, tag="dsb")
                    nc.sync.dma_start(
                        out=dsb[:, :nq * PPIX],
                        in_=din[:, p0:p0 + nq].rearrange("p g f -> p (g f)"))
                    osb = opool.tile([Q, DQUAD * PIX], BF16, tag="osb")

                    for gl in range(nq):
                        g = p0 + gl
                        gi = g - g0
                        drows = dsb[:, gl * PPIX:(gl + 1) * PPIX].rearrange(
                            "p (r c) -> p r c", c=PADW)
                        pm0 = pmpool.tile([QP, HALF], F32, tag="pm")
                        pm1 = pmpool.tile([QP, HALF], F32, tag="pm")
                        pms = [pm0, pm1]
                        for t_ in range(KS * KS):
                            ky, kx = divmod(t_, KS)
                            lhsT = bd[:, gi, t_, :]
                            for h in range(2):
                                rhs = drows[:, h * 12 + ky:h * 12 + ky + 12,
                                            kx:kx + P]
                                nc.tensor.matmul(
                                    pms[h], lhsT=lhsT, rhs=rhs,
                                    start=(t_ == 0), stop=(t_ == KS * KS - 1),
                                    skip_group_check=True)

                        for h in range(2):
                            nc.scalar.activation(
                                out=osb[:, gl * PIX + h * HALF:
                                        gl * PIX + (h + 1) * HALF],
                                in_=pms[h][0:Q],
                                func=mybir.ActivationFunctionType.Copy,
                                bias=0.0, scale=r2[:, g:g + 1])
                    oeng = nc.gpsimd if (p0 // DQUAD) % 2 == 0 else nc.sync
                    oeng.dma_start(
                        out=outd[:, p0:p0 + nq].rearrange("p g f -> p (g f)"),
                        in_=osb[:, :nq * PIX])

                g0 += ng

    nc.compile()
    return nc


def _host_prep(guidance, depth, conv_w, conv_b, dense_w, dense_b):
    B, H, W, _ = guidance.shape
    nh, nw = H // P, W // P
    NB = B * nh * nw

    def to_samples(x):
        # (B,H,W,F) -> (NB, P, P, F), sample order = flat (b, i, j)
        return (x.reshape(B, nh, P, nw, P, F)
                 .transpose(0, 1, 3, 2, 4, 5)
                 .reshape(NB, P, P, F))

    gs = to_samples(np.ascontiguousarray(guidance))
    ds = to_samples(np.ascontiguousarray(depth))

    in_maps = []
    for c in range(NCORES):
        gsl = gs[c * 512:(c + 1) * 512]
        dsl = ds[c * 512:(c + 1) * 512]
        gpad = np.zeros((SPC, P, P, F), BF)
        gpad[:512] = gsl
        dpad = np.zeros((SPC, PADW, PADW, F), BF)
        dpad[:512, 1:P + 1, 1:P + 1] = dsl
        # (SPC, y, x, ch) -> [NGROUP, 126, pix]  with q = n_local*9 + ch
        gq = (gpad.reshape(NGROUP, NL, P, P, F)
                  .transpose(1, 4, 0, 2, 3)
                  .reshape(Q, NGROUP, PIX))
        dq = (dpad.reshape(NGROUP, NL, PADW, PADW, F)
                  .transpose(1, 4, 0, 2, 3)
                  .reshape(Q, NGROUP, PPIX))
        in_maps.append({"gin": np.ascontiguousarray(gq),
                        "din": np.ascontiguousarray(dq)})

    # compact per-partition weights; block-diag expansion happens on-chip
    iofq = np.arange(Q) % F                       # i(q)
    lhsAc = np.ascontiguousarray(
        conv_w.reshape(KS * KS, F, F)[:, iofq, :].transpose(1, 0, 2)).astype(BF)
    dws = dense_w.astype(np.float32) / PIX  # gap arrives as a SUM over pixels
    lhsDc = np.zeros((Q + 1, F, F), np.float32)
    lhsD2c = np.zeros((Q + 1, F, F), np.float32)
    for j in range(F):
        lhsDc[:Q, j, :] = dws[iofq][:, j * F:(j + 1) * F]
        lhsDc[Q, j, :] = dense_b[j * F:(j + 1) * F]
        lhsD2c[:Q, j, :] = dws[iofq][:, j::F]
        lhsD2c[Q, j, :] = dense_b[j::F]
    maskE = np.zeros((Q + 1, Q), np.float32)
    maskE[:Q] = np.kron(np.eye(NL, dtype=np.float32), np.ones((F, F), np.float32))
    maskE[Q] = 1.0
    convb = np.tile(conv_b.astype(np.float32), NL)[:, None]

    cpack = np.zeros((Q + 1, 3 * F * F + Q + 1), np.float32)
    cpack[:Q, 0:F * F] = np.asarray(lhsAc, np.float32).reshape(Q, F * F)
    cpack[:, F * F:2 * F * F] = lhsDc.reshape(Q + 1, F * F)
    cpack[:, 2 * F * F:3 * F * F] = lhsD2c.reshape(Q + 1, F * F)
    cpack[:, 3 * F * F:3 * F * F + Q] = maskE
    cpack[:Q, 3 * F * F + Q] = convb[:, 0]
    consts = {"cpack": np.ascontiguousarray(cpack.astype(BF))}
    for m in in_maps:
        m.update(consts)
    return in_maps


_CACHED_NC = None


def run(inputs, trace=False, **kw):
    """Build (cached), run on 8 cores, return (full_output, BassKernelResults)."""
    global _CACHED_NC
    inputs = {k: np.asarray(v, np.float32) for k, v in inputs.items()}
    in_maps = _host_prep(**inputs)
    if _CACHED_NC is None:
        _CACHED_NC = build_program()
    res = run_bass_kernel_spmd(_CACHED_NC, in_maps, list(range(NCORES)),
                               trace=trace, **kw)
    outs = []
    for c in range(NCORES):
        o = res.results[c]["out"].astype(np.float32)
        o = o.reshape(NL, F, NGROUP, P, P)
        o = o.transpose(2, 0, 3, 4, 1).reshape(SPC, P, P, F)[:512]
        outs.append(o)
    full = np.concatenate(outs, 0)  # (4096, 24, 24, 9) in (b, i, j) order
    B, H, W = 16, 384, 384
    return full.reshape(B, H, W, F), res


def kernel(**inputs):
    out, _ = run(inputs, trace=False)
    return out
